# revision 15
# baseline (speedup 1.0000x reference)
"""Trainium2 Bass kernel for ComplementaryChannelInteraction.

Pipeline (per sample):
  1x1 conv (+folded BN1) -> ReLU -> channel attention softmax(-x@xT) ->
  3x3 conv (+folded BN2) -> ReLU -> global avg pool -> FC -> ReLU -> FC

Sharding: pure data parallel, B=128 -> 16 samples on each of 8 cores,
all params replicated.

Structure (v2):
  - conv1 is channel-major and pair-batched: weights stationary, a
    sample-pair's features stream as N=392, producing x channel-major
    (xcm) directly.  Spatial-major fT is built from xcm with 8 PE
    transposes per sample (needed for the x@xT contraction over p).
  - softmax uses the xxt symmetry: G[d,c] = exp(m_c - xxt[d,c]) is
    E^T, computed directly in the [d, c] layout the y-matmul needs as
    its stationary operand -- no E transposes.  The per-row shift m_c
    is folded into the xxt PSUM accumulation as a rank-1 matmul
    (-ones x m_row), so exp() needs no free-dim bias.  The shift
    cancels exactly between numerator and denominator, so m only
    needs to be range-correct (bf16 is fine).
  - Z_c arrives for free as column 196 of the y-matmul (ones column
    appended to xcm).
  - conv3 runs in fp8e4 (w3 scaled by a global power of two, descaled in
    the pooling activation) with perf_mode=DoubleRow pairing ci-chunks,
    each weight tile streamed over 8 samples, two samples sharing each
    PSUM bank (single bank-clearing start, everything else accumulates
    or first-touch-overwrites via has_written).
"""
import contextlib
import ctypes
import sys
import types

import numpy as np
import ml_dtypes

import concourse.bass as bass
import concourse.tile as tile
import concourse.mybir as mybir
from concourse import bacc
from concourse.bass_utils import run_bass_kernel_spmd

dt = mybir.dt
F32, F32R, BF16 = dt.float32, dt.float32r, dt.bfloat16
FP8 = dt.float8e4
PM = mybir.MatmulPerfMode
AF = mybir.ActivationFunctionType
ALU = mybir.AluOpType
AX = mybir.AxisListType

N_CORES = 8
B, CIN, C, H, W, NCOUT = 128, 2048, 512, 14, 14, 200
P = H * W            # 196
BPC = B // N_CORES   # 16 samples per core
KC = CIN // 128      # 16 contraction chunks for conv1
CC = C // 128        # 4 channel chunks
PCH = [(0, 128), (128, 68)]  # spatial chunks of 196: (offset, size)
EPS = 1e-5
ALPHA = 128.0        # global fp8 scale for conv3 weights

# ---------------------------------------------------------------- compat shims


def _install_drain_patch():
    """walrus here allows only ONE sync-wait per Drain; split the Tile
    kernel-tail drain into a chain of single-wait drains."""

    def _split_drain_and_barrier(self, tick_clock, wait_clock):
        from concourse.tile import ScopedClock

        drain_inst = self.nc.sync.drain()
        wait_clock.add_sem_waits(
            drain_inst.ins, ScopedClock({None: tick_clock.global_clock})
        )
        si = drain_inst.ins.sync_info
        waits = list(si.on_wait) if si is not None else []
        if len(waits) > 1:
            drain_inst.ins.sync_info = mybir.SyncInfo(
                on_wait=waits[:1], on_update=list(si.on_update)
            )
            for i in range(1, len(waits)):
                extra = self.nc.sync.drain()
                extra.ins.sync_info = mybir.SyncInfo(
                    on_wait=waits[i : i + 1], on_update=[]
                )
        self.nc.all_engine_barrier()
        assert self.sems is not None
        popped = self.nc._tile_sem_poison_stack.pop()
        assert popped is self._sem_poison
        self.nc.clear_and_free_semaphores(list(self.sems.allocated().values()))
        self.nc.all_engine_barrier()

    tile.TileContext._drain_and_barrier = _split_drain_and_barrier


def _install_ntff_hook(so_path="/opt/axon/libaxon_pjrt.so"):
    """antenv.axon_hooks is missing in this image; recreate it so
    trace=True (NTFF profiling) works instead of crashing on import."""
    if "antenv.axon_hooks" in sys.modules:
        return
    mod = types.ModuleType("antenv.axon_hooks")
    state = {"hook": None}
    mod.set_axon_ntff_profile_hook = lambda h: state.__setitem__("hook", h)
    mod.get_axon_ntff_profile_hook = lambda: state["hook"]
    sys.modules["antenv.axon_hooks"] = mod
    try:
        import antenv

        antenv.axon_hooks = mod
    except ImportError:
        pass
    try:
        lib = ctypes.CDLL(so_path)
        if not hasattr(lib, "axon_start_nrt_profile"):
            return
        lib.axon_start_nrt_profile.argtypes = [
            ctypes.POINTER(ctypes.c_int64),
            ctypes.c_size_t,
        ]
        lib.axon_start_nrt_profile.restype = ctypes.c_int64
        lib.axon_stop_nrt_profile.argtypes = [ctypes.c_char_p]
        lib.axon_stop_nrt_profile.restype = ctypes.c_int64
    except OSError:
        return

    @contextlib.contextmanager
    def _hook(output_dir, device_ids):
        import jax

        jax.devices()
        if device_ids:
            ids = (ctypes.c_int64 * len(device_ids))(*device_ids)
            rc = lib.axon_start_nrt_profile(ids, len(device_ids))
        else:
            rc = lib.axon_start_nrt_profile(None, 0)
        if rc != 0:
            raise RuntimeError(f"axon_start_nrt_profile rc={rc}")
        try:
            yield
        finally:
            n = lib.axon_stop_nrt_profile(str(output_dir).encode())
            if n < 0:
                raise RuntimeError(f"axon_stop_nrt_profile rc={n}")
            print(f"profile: {n} file(s) written to {output_dir}", file=sys.stderr)

    state["hook"] = _hook


def install_shims():
    _install_drain_patch()
    _install_ntff_hook()


# ---------------------------------------------------------------- bass program


def build_program(n_samples=BPC):
    install_shims()
    nc = bacc.Bacc(
        "TRN2", target_bir_lowering=False, debug=False, num_devices=N_CORES
    )

    n_pairs = n_samples // 2
    feat_d = nc.dram_tensor("feat", [n_samples // 2, KC, 128, 2, P], BF16, kind="ExternalInput")
    wpT_d = nc.dram_tensor("wpT", [CIN, C], BF16, kind="ExternalInput")
    t1_d = nc.dram_tensor("t1b", [CC, 128], F32, kind="ExternalInput")
    w3b_d = nc.dram_tensor("w3b", [128, 2, 2, 9, C], FP8, kind="ExternalInput")
    t2_d = nc.dram_tensor("t2", [CC, 128], F32, kind="ExternalInput")
    fc1_d = nc.dram_tensor("fc1", [CC, 128, NCOUT], F32, kind="ExternalInput")
    fc1b_d = nc.dram_tensor("fc1b", [2, 100], F32, kind="ExternalInput")
    fc2_d = nc.dram_tensor("fc2", [2, 100, NCOUT], F32, kind="ExternalInput")
    fc2b_d = nc.dram_tensor("fc2b", [2, 100], F32, kind="ExternalInput")
    ident_d = nc.dram_tensor("ident", [128, 128], F32, kind="ExternalInput")
    identb_d = nc.dram_tensor("identb", [128, 128], BF16, kind="ExternalInput")
    out_d = nc.dram_tensor("out", [n_samples, NCOUT], F32, kind="ExternalOutput")

    with tile.TileContext(nc) as tc:
        with contextlib.ExitStack() as ctx:
            wpool = ctx.enter_context(tc.tile_pool(name="weights", bufs=1))
            featp = ctx.enter_context(tc.tile_pool(name="featp", bufs=3 * KC))
            fTp = ctx.enter_context(tc.tile_pool(name="fTp", bufs=2))
            xcmp = ctx.enter_context(tc.tile_pool(name="xcmp", bufs=3))
            Gp = ctx.enter_context(tc.tile_pool(name="Gp", bufs=2))
            smallp = ctx.enter_context(tc.tile_pool(name="smallp", bufs=4))
            zscrp = ctx.enter_context(tc.tile_pool(name="zscrp", bufs=2))
            mbcp = ctx.enter_context(tc.tile_pool(name="mbcp", bufs=2))
            xsp = ctx.enter_context(tc.tile_pool(name="xsp", bufs=2))
            ps_conv = ctx.enter_context(tc.tile_pool(name="ps_conv", bufs=2, space="PSUM"))
            ps_big = ctx.enter_context(tc.tile_pool(name="ps_big", bufs=4, space="PSUM"))
            ps_sm = ctx.enter_context(tc.tile_pool(name="ps_sm", bufs=2, space="PSUM"))

            # ---- startup-critical loads: identb + t1 first (tiny), then
            # per-k wpT chunks interleaved with pair-0 feature chunks so the
            # first conv1 matmul only waits for ~700KB.
            identb = wpool.tile([128, 128], BF16, tag="identb")
            nc.sync.dma_start(identb[:], identb_d[:])
            t1sb = wpool.tile([128, CC], F32, tag="t1sb")
            nc.sync.dma_start(t1sb[:], t1_d[:].rearrange("j p -> p j"))

            feat_pend = {}

            def load_pair(p):
                tiles = []
                for k in range(KC):
                    ft = featp.tile([128, 2, P], BF16, tag="feat")
                    nc.sync.dma_start(ft[:], feat_d[p, k])
                    tiles.append(ft)
                feat_pend[p] = tiles
                return tiles

            wpT = []
            feat0 = []
            feat1 = []
            for k in range(KC):
                wt = wpool.tile([128, C], BF16, tag=f"wpT{k}")
                nc.sync.dma_start(wt[:, : C // 2], wpT_d[k * 128 : (k + 1) * 128, : C // 2])
                wpT.append(wt)
                ft = featp.tile([128, 2, P], BF16, tag="feat")
                nc.sync.dma_start(ft[:], feat_d[0, k])
                feat0.append(ft)
            for k in range(KC):
                nc.sync.dma_start(
                    wpT[k][:, C // 2 :], wpT_d[k * 128 : (k + 1) * 128, C // 2 :]
                )
                ft = featp.tile([128, 2, P], BF16, tag="feat")
                nc.sync.dma_start(ft[:], feat_d[1, k])
                feat1.append(ft)

            ones1 = wpool.tile([1, 128], BF16, tag="ones1")
            nc.vector.memset(ones1[:], 1.0)

            # persistent padded-y tiles, one per pair, 4 pairs per conv3
            # octet.  Borders stay zero across reuse (y writes interior only).
            ypads = []
            for g in range(4):
                yp = wpool.tile([128, 2, 2, 2, 16, 16], FP8, tag=f"ypad{g}")
                nc.vector.memset(yp[:], 0.0)
                ypads.append(yp)

            # accumulated pooled z for the whole per-core batch
            zall = wpool.tile([128, CC, n_samples], F32, tag="zall")

            state = {"w3sb": None, "t2sb": None, "fc1sb": None,
                     "fc1bsb": None, "fc2sb": None, "fc2bsb": None}

            def emit_w3_loads():
                state["w3sb"] = wpool.tile([128, 2, 2, 9, C], FP8, tag="w3sb", name="w3sb")
                nc.sync.dma_start(state["w3sb"][:], w3b_d[:])
                state["t2sb"] = wpool.tile([128, CC], F32, tag="t2sb", name="t2sb")
                nc.sync.dma_start(state["t2sb"][:], t2_d[:].rearrange("j p -> p j"))

            def emit_fc_loads():
                state["fc1sb"] = wpool.tile([128, CC, NCOUT], F32, tag="fc1sb", name="fc1sb")
                nc.sync.dma_start(state["fc1sb"][:], fc1_d[:].rearrange("j p o -> p j o"))
                state["fc1bsb"] = wpool.tile([128, 2], F32, tag="fc1bsb", name="fc1bsb")
                nc.sync.dma_start(state["fc1bsb"][:100, :], fc1b_d[:].rearrange("m p -> p m"))
                state["fc2sb"] = wpool.tile([128, 2, NCOUT], F32, tag="fc2sb", name="fc2sb")
                nc.sync.dma_start(state["fc2sb"][:100, :, :], fc2_d[:].rearrange("m p o -> p m o"))
                state["fc2bsb"] = wpool.tile([128, 2], F32, tag="fc2bsb", name="fc2bsb")
                nc.sync.dma_start(state["fc2bsb"][:100, :], fc2b_d[:].rearrange("m p -> p m"))

            class Conv1Weave:
                """channel-major 1x1 conv for a sample pair, split into
                emit()-able slices so the matmuls can be woven into the
                previous pair's attention stalls.  Produces (xcmA, xcmB),
                each [128, CC, P+1] bf16 with a ones column at [.., P]."""

                def __init__(self, feat):
                    self.feat = feat
                    self.xcms = []
                    for v in range(2):
                        xcm = xcmp.tile([128, CC, P + 1], BF16, tag="xcm", name="xcm")
                        nc.vector.memset(xcm[:, :, P : P + 1], 1.0)
                        self.xcms.append(xcm)
                    self.thunks = []
                    for half in range(2):
                        pcs_box = {}

                        def alloc(half=half, pcs_box=pcs_box):
                            pcs_box["pcs"] = [
                                ps_conv.tile([128, 2, P], F32, tag="conv", name="pc")
                                for _ in range(2)
                            ]

                        for ks in range(0, KC, 4):
                            def mm_slice(ks=ks, half=half, pcs_box=pcs_box, alloc=alloc):
                                if ks == 0:
                                    alloc()
                                pcs = pcs_box["pcs"]
                                for k in range(ks, ks + 4):
                                    for ii in range(2):
                                        i = 2 * half + ii
                                        nc.tensor.matmul(
                                            pcs[ii][:],
                                            wpT[k][:, i * 128 : (i + 1) * 128],
                                            self.feat[k][:],
                                            start=(k == 0),
                                            stop=(k == KC - 1),
                                        )
                            self.thunks.append(mm_slice)

                        def acts(half=half, pcs_box=pcs_box):
                            pcs = pcs_box["pcs"]
                            for ii in range(2):
                                i = 2 * half + ii
                                for v in range(2):
                                    nc.scalar.activation(
                                        self.xcms[v][:, i, :P],
                                        pcs[ii][:, v, :],
                                        AF.Relu,
                                        bias=t1sb[:, i : i + 1],
                                    )
                        self.thunks.append(acts)
                    self._next = 0

                def emit(self, n):
                    for t in self.thunks[self._next : self._next + n]:
                        t()
                    self._next += n

                def emit_rest(self):
                    self.emit(len(self.thunks) - self._next)

            def transp_sample(xcm):
                """spatial-major fT [p(2 chunks), c] from channel-major xcm."""
                fT = fTp.tile([128, 2, C], BF16, tag="fT")
                for i in range(CC):
                    ptr = ps_sm.tile([128, 2, 128], BF16, tag="sm")
                    for m, (po, pn) in enumerate(PCH):
                        nc.tensor.transpose(
                            ptr[:pn, m, :],
                            xcm[:, i, po : po + pn],
                            identb[:, :],
                        )
                    nc.vector.tensor_copy(fT[:, :, i * 128 : (i + 1) * 128], ptr[:])
                return fT

            def xxt_mms(fT):
                """xxt chunks [d(chunk i), all c] in PSUM."""
                pxxts = []
                for i in range(CC):
                    px = ps_big.tile([128, C], F32, tag="big")
                    for m, (po, pn) in enumerate(PCH):
                        nc.tensor.matmul(
                            px[:],
                            fT[:pn, m, i * 128 : (i + 1) * 128],
                            fT[:pn, m, :],
                            start=(m == 0),
                            stop=(m == 1),
                        )
                    pxxts.append(px)
                return pxxts

            def xxt_mins(pxxts):
                mrow = smallp.tile([128, CC], BF16, tag="mrow")
                for i in range(CC):
                    nc.vector.tensor_reduce(
                        out=mrow[:, i : i + 1], in_=pxxts[i][:], op=ALU.min, axis=AX.X
                    )
                return mrow

            def xxt_sample(fT):
                pxxts = xxt_mms(fT)
                return pxxts, xxt_mins(pxxts)

            def shift_part1(mrow):
                """broadcast m along the free dim: transpose columns of mrow
                to a single partition row, then ones outer product."""
                mT1 = ps_sm.tile([1, CC, 128], BF16, tag="sm")
                for j in range(CC):
                    nc.tensor.transpose(
                        mT1[0:1, j, :], mrow[:, j : j + 1], identb[:, :]
                    )
                m_row = smallp.tile([1, C], BF16, tag="mline")
                nc.vector.tensor_copy(m_row[0:1, :], mT1[0:1, :, :])
                pmb = ps_sm.tile([128, C], F32, tag="sm")
                nc.tensor.matmul(pmb[:], ones1[0:1, :], m_row[0:1, :])
                return pmb

            def shift_part2(pxxts, pmb):
                """xs = m_c - xxt on DVE (freeing the xxt banks), then
                G = exp(xs) in bf16.  No PE work -- callers put PE filler
                after this emission."""
                mbc = mbcp.tile([128, C], F32, tag="mbc")
                nc.vector.tensor_copy(mbc[:], pmb[:])
                xs = xsp.tile([128, CC, C], F32, tag="xs")
                G = Gp.tile([128, CC, C], BF16, tag="G")
                for i in range(CC):
                    nc.vector.scalar_tensor_tensor(
                        out=xs[:, i, :],
                        in0=pxxts[i][:],
                        scalar=-1.0,
                        in1=mbc[:],
                        op0=ALU.mult,
                        op1=ALU.add,
                    )
                    nc.scalar.activation(G[:, i, :], xs[:, i, :], AF.Exp)
                return G

            def y_sample(G, xcm, ypad, v):
                """y = (E @ x) / Z into ypad interior; Z from ones column."""
                zinv = smallp.tile([128, CC], F32, tag="zinv")
                for i in range(CC):
                    py = ps_sm.tile([128, P + 1], F32, tag="sm")
                    for j in range(CC):
                        nc.tensor.matmul(
                            py[:],
                            G[:, j, i * 128 : (i + 1) * 128],
                            xcm[:, j, :],
                            start=(j == 0),
                            stop=(j == CC - 1),
                        )
                    nc.vector.reciprocal(zinv[:, i : i + 1], py[:, P : P + 1])
                    nc.scalar.activation(
                        ypad[:, i // 2, i % 2, v, 1 : 1 + H, 1 : 1 + W],
                        py[:, :P].rearrange("p (h w) -> p h w", h=H),
                        AF.Copy,
                        scale=zinv[:, i : i + 1],
                    )

            def conv3_octet(pairs, fc_hook=None):
                """3x3 conv (fp8 DoubleRow over ci-chunk pairs) + BN2 + ReLU
                + spatial sum.  Each weight tile streams over all samples;
                two samples share one PSUM bank: only the very first matmul
                into a bank uses start=True (bank clear), the other sample's
                first matmul relies on has_written first-touch overwrite."""
                w3sb, t2sb = state["w3sb"], state["t2sb"]
                ng = len(pairs)
                for i in range(CC):
                    pzs = [ps_big.tile([128, 2, H, W], F32, tag="big", name=f"pz{g}") for g in range(ng)]
                    for t, (ky, kx) in enumerate(
                        (ky, kx) for ky in range(3) for kx in range(3)
                    ):
                        if fc_hook is not None and i > 0 and t == 3:
                            fc_hook(i - 1)
                        for jp in range(2):
                            for g in range(ng):
                                for v in range(2):
                                    nc.tensor.matmul(
                                        pzs[g][:, v],
                                        w3sb[:, jp, :, t, i * 128 : (i + 1) * 128],
                                        ypads[g][:, jp, :, v, ky : ky + H, kx : kx + W],
                                        start=(t == 0 and jp == 0 and v == 0),
                                        stop=(t == 8 and jp == 1),
                                        perf_mode=PM.DoubleRow,
                                        skip_group_check=True,
                                    )
                    for g in range(ng):
                        for v in range(2):
                            s = 2 * pairs[g] + v
                            zscr = zscrp.tile([128, H, W], BF16, tag="zscr")
                            nc.scalar.activation(
                                zscr[:],
                                pzs[g][:, v],
                                AF.Relu,
                                bias=t2sb[:, i : i + 1],
                                scale=float(1.0 / ALPHA),
                                accum_out=zall[:, i, s : s + 1],
                            )
                    if fc_hook is not None and i == CC - 1:
                        fc_hook(i)

            # ---------------- main pipeline over pairs
            feat_pend[1] = feat1
            wv0 = Conv1Weave(feat0)
            wv0.emit_rest()
            xcm_cur = wv0.xcms
            for p in range(n_pairs):
                xcmA, xcmB = xcm_cur
                if p + 2 < n_pairs:
                    load_pair(p + 2)
                if p == 2:
                    emit_w3_loads()
                if p == 4:
                    emit_fc_loads()
                wv = Conv1Weave(feat_pend.pop(p + 1)) if p + 1 < n_pairs else None
                ypad = ypads[p % 4]
                fTA = transp_sample(xcmA)
                pxA, mrA = xxt_sample(fTA)
                fTB = transp_sample(xcmB)        # PE filler while mins(A) land
                pmbA = shift_part1(mrA)
                GA = shift_part2(pxA, pmbA)
                pxB = xxt_mms(fTB)               # banks free as stts(A) land
                y_sample(GA, xcmA, ypad, 0)
                mrB = xxt_mins(pxB)              # after yA's recip/scale on DVE
                if wv:
                    wv.emit(5)                   # PE filler: mins(B) + exps(A)
                pmbB = shift_part1(mrB)
                GB = shift_part2(pxB, pmbB)
                if wv:
                    wv.emit_rest()               # PE filler: mbc/stt/exp(B)
                y_sample(GB, xcmB, ypad, 1)
                last_octet = p == n_pairs - 1
                if p % 4 == 3 or last_octet:
                    hook = None
                    if last_octet:
                        fc1sb = state["fc1sb"]
                        phs = [ps_sm.tile([128, n_samples], F32, tag="sm", name="ph")
                               for _ in range(2)]

                        def hook(j, phs=phs, fc1sb=fc1sb):
                            for m in range(2):
                                nc.tensor.matmul(
                                    phs[m][:100, :],
                                    fc1sb[:, j, m * 100 : (m + 1) * 100],
                                    zall[:, j, :],
                                    start=(j == 0),
                                    stop=(j == CC - 1),
                                )
                    conv3_octet(list(range((p // 4) * 4, p + 1)), fc_hook=hook)
                xcm_cur = wv.xcms if wv else None

            # ---- FC head over the whole per-core batch
            fc1bsb = state["fc1bsb"]
            fc2sb, fc2bsb = state["fc2sb"], state["fc2bsb"]
            h_sb = smallp.tile([128, 2, n_samples], F32, tag="h")
            for m in range(2):
                nc.scalar.activation(
                    h_sb[:100, m, :], phs[m][:100, :], AF.Relu, bias=fc1bsb[:100, m : m + 1]
                )
            sf_sb = smallp.tile([128, 2, n_samples], F32, tag="sf")
            for m2 in range(2):
                psf = ps_sm.tile([128, n_samples], F32, tag="sm")
                for m in range(2):
                    nc.tensor.matmul(
                        psf[:100, :],
                        fc2sb[:100, m, m2 * 100 : (m2 + 1) * 100],
                        h_sb[:100, m, :],
                        start=(m == 0),
                        stop=(m == 1),
                    )
                nc.scalar.activation(
                    sf_sb[:100, m2, :],
                    psf[:100, :],
                    AF.Identity,
                    bias=fc2bsb[:100, m2 : m2 + 1],
                )
                nc.sync.dma_start(
                    out_d[:, m2 * 100 : (m2 + 1) * 100].rearrange("b o -> o b"),
                    sf_sb[:100, m2, :],
                )

    nc.compile()
    return nc


# ---------------------------------------------------------------- host wrapper

_prog_cache = {}


def _get_program(n_samples=BPC):
    key = n_samples
    if key not in _prog_cache:
        _prog_cache[key] = build_program(n_samples)
    return _prog_cache[key]


def _fp8_tapsum_round(wa):
    """Quantize conv3 weights [co, ci, 3, 3] to fp8 e4m3 choosing per-tap
    rounding direction so the 9-tap sum error cancels per (co, ci).  The
    global-avg-pool output depends (to first order) on sum_tap w, so this
    kills the dominant correlated quantization term."""
    fp8 = ml_dtypes.float8_e4m3
    q = wa.astype(fp8)
    qf = q.astype(np.float32)
    bits = q.view(np.uint8).astype(np.int16)
    above = np.where(qf >= 0, bits + 1, bits - 1)
    below = np.where(qf >= 0, bits - 1, bits + 1)
    tgt = np.where(qf > wa, below, above).astype(np.int16)
    tgt = np.clip(tgt, 0, 255).astype(np.uint8)
    of = tgt.view(fp8).astype(np.float32)
    of = np.where(~np.isfinite(of), qf, of)
    co, ci = wa.shape[:2]
    Qf = qf.reshape(co, ci, 9).copy()
    D = (of - qf).reshape(co, ci, 9).copy()
    r = (qf - wa).reshape(co, ci, 9).sum(-1)
    for _ in range(4):
        cand = np.abs(r[..., None] + D)
        best = cand.argmin(-1)
        bi = np.take_along_axis(D, best[..., None], axis=-1)[..., 0]
        improve = np.abs(r + bi) < np.abs(r) - 1e-9
        bsel = np.where(improve, best, -1)
        for t in range(9):
            m = bsel == t
            Qf[m, t] = Qf[m, t] + D[m, t]
            r[m] += D[m, t]
            D[m, t] = -D[m, t]
    return Qf.reshape(co, ci, 9).astype(fp8)


def prepare_host_inputs(inputs):
    """Fold BN into weights, build the per-core replicated param arrays."""
    s1 = inputs["bn1_gamma"] / np.sqrt(inputs["bn1_var"] + EPS)
    t1 = (inputs["b_reduce"] - inputs["bn1_mean"]) * s1 + inputs["bn1_beta"]
    Wp = inputs["w_reduce"].reshape(C, CIN) * s1[:, None]
    wpT = np.ascontiguousarray(Wp.T).astype(ml_dtypes.bfloat16)  # [2048, 512]
    t1b = np.ascontiguousarray(t1.astype(np.float32).reshape(CC, 128))

    s2 = inputs["bn2_gamma"] / np.sqrt(inputs["bn2_var"] + EPS)
    t2 = (inputs["b3"] - inputs["bn2_mean"]) * s2 + inputs["bn2_beta"]
    w3p = inputs["w3"] * s2[:, None, None, None]            # [co, ci, ky, kx]
    w3q = _fp8_tapsum_round(np.asarray(w3p, np.float32) * ALPHA)
    # -> [ci_in(128), jp(2), jj(2), tap(9), co] fp8, scaled by ALPHA
    w3r = w3q.reshape(C, C, 9).transpose(1, 2, 0).reshape(2, 2, 128, 9, C)
    w3b = np.ascontiguousarray(w3r.transpose(2, 0, 1, 3, 4))
    t2_a = np.ascontiguousarray(t2.reshape(CC, 128))

    fc1p = (inputs["fc1_w"] / float(P)).astype(np.float32)  # fold 1/196 mean
    fc1 = np.ascontiguousarray(fc1p.T.reshape(CC, 128, NCOUT))
    fc1b = np.ascontiguousarray(inputs["fc1_b"].reshape(2, 100))
    fc2 = np.ascontiguousarray(inputs["fc2_w"].T.reshape(2, 100, NCOUT))
    fc2b = np.ascontiguousarray(inputs["fc2_b"].reshape(2, 100))
    ident = np.eye(128, dtype=np.float32)
    identb = np.eye(128, dtype=ml_dtypes.bfloat16)
    return {
        "identb": identb,
        "wpT": wpT,
        "t1b": t1b,
        "w3b": w3b,
        "t2": t2_a,
        "fc1": fc1,
        "fc1b": fc1b,
        "fc2": fc2,
        "fc2b": fc2b,
        "ident": ident,
    }


def run(inputs, n_samples=BPC, n_cores=N_CORES, trace=False):
    nc = _get_program(n_samples)
    params = prepare_host_inputs(inputs)
    feat = np.asarray(inputs["feature"], np.float32).reshape(B, CIN, P).astype(ml_dtypes.bfloat16)
    feat = feat.reshape(B // 2, 2, KC, 128, P).transpose(0, 2, 3, 1, 4)
    npair = n_samples // 2
    in_maps = []
    for c in range(n_cores):
        m = dict(params)
        m["feat"] = np.ascontiguousarray(feat[c * npair : (c + 1) * npair])
        in_maps.append(m)
    res = run_bass_kernel_spmd(nc, in_maps, list(range(n_cores)), trace=trace)
    out = np.concatenate([res.results[c]["out"] for c in range(n_cores)], axis=0)
    return out, res


def kernel(**inputs):
    inputs = {k: np.asarray(v) for k, v in inputs.items()}
    out, _ = run(inputs)
    return out.astype(np.float32)


# revision 20
# speedup vs baseline: 1.0173x; 1.0173x over previous
"""Trainium2 Bass kernel for ComplementaryChannelInteraction.

Pipeline (per sample):
  1x1 conv (+folded BN1) -> ReLU -> channel attention softmax(-x@xT) ->
  3x3 conv (+folded BN2) -> ReLU -> global avg pool -> FC -> ReLU -> FC

Sharding: pure data parallel, B=128 -> 16 samples on each of 8 cores,
all params replicated.

Structure (v2):
  - conv1 is channel-major and pair-batched: weights stationary, a
    sample-pair's features stream as N=392, producing x channel-major
    (xcm) directly.  Spatial-major fT is built from xcm with 8 PE
    transposes per sample (needed for the x@xT contraction over p).
  - softmax uses the xxt symmetry: G[d,c] = exp(m_c - xxt[d,c]) is
    E^T, computed directly in the [d, c] layout the y-matmul needs as
    its stationary operand -- no E transposes.  The per-row shift m_c
    is folded into the xxt PSUM accumulation as a rank-1 matmul
    (-ones x m_row), so exp() needs no free-dim bias.  The shift
    cancels exactly between numerator and denominator, so m only
    needs to be range-correct (bf16 is fine).
  - Z_c arrives for free as column 196 of the y-matmul (ones column
    appended to xcm).
  - conv3 runs in fp8e4 (w3 scaled by a global power of two, descaled in
    the pooling activation) with perf_mode=DoubleRow pairing ci-chunks,
    each weight tile streamed over 8 samples, two samples sharing each
    PSUM bank (single bank-clearing start, everything else accumulates
    or first-touch-overwrites via has_written).
"""
import contextlib
import ctypes
import sys
import types

import numpy as np
import ml_dtypes

import concourse.bass as bass
import concourse.tile as tile
import concourse.mybir as mybir
from concourse import bacc
from concourse.bass_utils import run_bass_kernel_spmd

dt = mybir.dt
F32, F32R, BF16 = dt.float32, dt.float32r, dt.bfloat16
FP8 = dt.float8e4
PM = mybir.MatmulPerfMode
AF = mybir.ActivationFunctionType
ALU = mybir.AluOpType
AX = mybir.AxisListType

N_CORES = 8
B, CIN, C, H, W, NCOUT = 128, 2048, 512, 14, 14, 200
P = H * W            # 196
BPC = B // N_CORES   # 16 samples per core
KC = CIN // 128      # 16 contraction chunks for conv1
CC = C // 128        # 4 channel chunks
PCH = [(0, 128), (128, 68)]  # spatial chunks of 196: (offset, size)
EPS = 1e-5
ALPHA = 128.0        # global fp8 scale for conv3 weights

# ---------------------------------------------------------------- compat shims


def _install_drain_patch():
    """walrus here allows only ONE sync-wait per Drain; split the Tile
    kernel-tail drain into a chain of single-wait drains."""

    def _split_drain_and_barrier(self, tick_clock, wait_clock):
        from concourse.tile import ScopedClock

        drain_inst = self.nc.sync.drain()
        wait_clock.add_sem_waits(
            drain_inst.ins, ScopedClock({None: tick_clock.global_clock})
        )
        si = drain_inst.ins.sync_info
        waits = list(si.on_wait) if si is not None else []
        if len(waits) > 1:
            drain_inst.ins.sync_info = mybir.SyncInfo(
                on_wait=waits[:1], on_update=list(si.on_update)
            )
            for i in range(1, len(waits)):
                extra = self.nc.sync.drain()
                extra.ins.sync_info = mybir.SyncInfo(
                    on_wait=waits[i : i + 1], on_update=[]
                )
        self.nc.all_engine_barrier()
        assert self.sems is not None
        popped = self.nc._tile_sem_poison_stack.pop()
        assert popped is self._sem_poison
        self.nc.clear_and_free_semaphores(list(self.sems.allocated().values()))
        self.nc.all_engine_barrier()

    tile.TileContext._drain_and_barrier = _split_drain_and_barrier


def _install_ntff_hook(so_path="/opt/axon/libaxon_pjrt.so"):
    """antenv.axon_hooks is missing in this image; recreate it so
    trace=True (NTFF profiling) works instead of crashing on import."""
    if "antenv.axon_hooks" in sys.modules:
        return
    mod = types.ModuleType("antenv.axon_hooks")
    state = {"hook": None}
    mod.set_axon_ntff_profile_hook = lambda h: state.__setitem__("hook", h)
    mod.get_axon_ntff_profile_hook = lambda: state["hook"]
    sys.modules["antenv.axon_hooks"] = mod
    try:
        import antenv

        antenv.axon_hooks = mod
    except ImportError:
        pass
    try:
        lib = ctypes.CDLL(so_path)
        if not hasattr(lib, "axon_start_nrt_profile"):
            return
        lib.axon_start_nrt_profile.argtypes = [
            ctypes.POINTER(ctypes.c_int64),
            ctypes.c_size_t,
        ]
        lib.axon_start_nrt_profile.restype = ctypes.c_int64
        lib.axon_stop_nrt_profile.argtypes = [ctypes.c_char_p]
        lib.axon_stop_nrt_profile.restype = ctypes.c_int64
    except OSError:
        return

    @contextlib.contextmanager
    def _hook(output_dir, device_ids):
        import jax

        jax.devices()
        if device_ids:
            ids = (ctypes.c_int64 * len(device_ids))(*device_ids)
            rc = lib.axon_start_nrt_profile(ids, len(device_ids))
        else:
            rc = lib.axon_start_nrt_profile(None, 0)
        if rc != 0:
            raise RuntimeError(f"axon_start_nrt_profile rc={rc}")
        try:
            yield
        finally:
            n = lib.axon_stop_nrt_profile(str(output_dir).encode())
            if n < 0:
                raise RuntimeError(f"axon_stop_nrt_profile rc={n}")
            print(f"profile: {n} file(s) written to {output_dir}", file=sys.stderr)

    state["hook"] = _hook


def install_shims():
    _install_drain_patch()
    _install_ntff_hook()


# ---------------------------------------------------------------- bass program


def build_program(n_samples=BPC):
    install_shims()
    nc = bacc.Bacc(
        "TRN2", target_bir_lowering=False, debug=False, num_devices=N_CORES
    )

    n_pairs = n_samples // 2
    feat_d = nc.dram_tensor("feat", [n_samples // 2, KC, 128, 2, P], BF16, kind="ExternalInput")
    wpT_d = nc.dram_tensor("wpT", [CIN, C], BF16, kind="ExternalInput")
    t1_d = nc.dram_tensor("t1b", [CC, 128], F32, kind="ExternalInput")
    w3b_d = nc.dram_tensor("w3b", [128, 2, 2, 9, C], FP8, kind="ExternalInput")
    t2_d = nc.dram_tensor("t2", [CC, 128], F32, kind="ExternalInput")
    fc1_d = nc.dram_tensor("fc1", [CC, 128, NCOUT], F32, kind="ExternalInput")
    fc1b_d = nc.dram_tensor("fc1b", [2, 100], F32, kind="ExternalInput")
    fc2_d = nc.dram_tensor("fc2", [2, 100, NCOUT], F32, kind="ExternalInput")
    fc2b_d = nc.dram_tensor("fc2b", [2, 100], F32, kind="ExternalInput")
    ident_d = nc.dram_tensor("ident", [128, 128], F32, kind="ExternalInput")
    identb_d = nc.dram_tensor("identb", [128, 128], BF16, kind="ExternalInput")
    out_d = nc.dram_tensor("out", [n_samples, NCOUT], F32, kind="ExternalOutput")

    with tile.TileContext(nc) as tc:
        with contextlib.ExitStack() as ctx:
            wpool = ctx.enter_context(tc.tile_pool(name="weights", bufs=1))
            featp = ctx.enter_context(tc.tile_pool(name="featp", bufs=3 * KC))
            fTp = ctx.enter_context(tc.tile_pool(name="fTp", bufs=2))
            xcmp = ctx.enter_context(tc.tile_pool(name="xcmp", bufs=3))
            Gp = ctx.enter_context(tc.tile_pool(name="Gp", bufs=2))
            smallp = ctx.enter_context(tc.tile_pool(name="smallp", bufs=4))
            zscrp = ctx.enter_context(tc.tile_pool(name="zscrp", bufs=2))
            mbcp = ctx.enter_context(tc.tile_pool(name="mbcp", bufs=2))
            xsp = ctx.enter_context(tc.tile_pool(name="xsp", bufs=2))
            ps_conv = ctx.enter_context(tc.tile_pool(name="ps_conv", bufs=2, space="PSUM"))
            ps_big = ctx.enter_context(tc.tile_pool(name="ps_big", bufs=4, space="PSUM"))
            ps_sm = ctx.enter_context(tc.tile_pool(name="ps_sm", bufs=2, space="PSUM"))

            # ---- startup-critical loads: identb + t1 first (tiny), then
            # per-k wpT chunks interleaved with pair-0 feature chunks so the
            # first conv1 matmul only waits for ~700KB.
            identb = wpool.tile([128, 128], BF16, tag="identb")
            nc.sync.dma_start(identb[:], identb_d[:])
            t1sb = wpool.tile([128, CC], F32, tag="t1sb")
            nc.sync.dma_start(t1sb[:], t1_d[:].rearrange("j p -> p j"))

            feat_pend = {}

            def load_pair(p):
                tiles = []
                for k in range(KC):
                    ft = featp.tile([128, 2, P], BF16, tag="feat")
                    nc.sync.dma_start(ft[:], feat_d[p, k])
                    tiles.append(ft)
                feat_pend[p] = tiles
                return tiles

            wpT = []
            feat0 = []
            feat1 = []
            for k in range(KC):
                wt = wpool.tile([128, C], BF16, tag=f"wpT{k}")
                nc.sync.dma_start(wt[:, : C // 2], wpT_d[k * 128 : (k + 1) * 128, : C // 2])
                wpT.append(wt)
                ft = featp.tile([128, 2, P], BF16, tag="feat")
                nc.sync.dma_start(ft[:], feat_d[0, k])
                feat0.append(ft)
            for k in range(KC):
                nc.sync.dma_start(
                    wpT[k][:, C // 2 :], wpT_d[k * 128 : (k + 1) * 128, C // 2 :]
                )
                ft = featp.tile([128, 2, P], BF16, tag="feat")
                nc.sync.dma_start(ft[:], feat_d[1, k])
                feat1.append(ft)

            ones1 = wpool.tile([1, 128], BF16, tag="ones1")
            nc.vector.memset(ones1[:], 1.0)
            zerot = wpool.tile([128, H, W], BF16, tag="zerot")
            nc.vector.memset(zerot[:], 0.0)

            # persistent padded-y tiles, one per pair, 4 pairs per conv3
            # octet.  Borders stay zero across reuse (y writes interior only).
            ypads = []
            for g in range(4):
                yp = wpool.tile([128, 2, 2, 2, 16, 16], FP8, tag=f"ypad{g}")
                nc.vector.memset(yp[:], 0.0)
                ypads.append(yp)

            # accumulated pooled z for the whole per-core batch
            zall = wpool.tile([128, CC, n_samples], F32, tag="zall")

            state = {"w3sb": None, "t2sb": None, "fc1sb": None,
                     "fc1bsb": None, "fc2sb": None, "fc2bsb": None}

            def emit_w3_loads():
                state["w3sb"] = wpool.tile([128, 2, 2, 9, C], FP8, tag="w3sb", name="w3sb")
                nc.sync.dma_start(state["w3sb"][:], w3b_d[:])
                state["t2sb"] = wpool.tile([128, CC], F32, tag="t2sb", name="t2sb")
                nc.sync.dma_start(state["t2sb"][:], t2_d[:].rearrange("j p -> p j"))

            def emit_fc_loads():
                state["fc1sb"] = wpool.tile([128, CC, NCOUT], F32, tag="fc1sb", name="fc1sb")
                nc.sync.dma_start(state["fc1sb"][:], fc1_d[:].rearrange("j p o -> p j o"))
                state["fc1bsb"] = wpool.tile([128, 2], F32, tag="fc1bsb", name="fc1bsb")
                nc.sync.dma_start(state["fc1bsb"][:100, :], fc1b_d[:].rearrange("m p -> p m"))
                state["fc2sb"] = wpool.tile([128, 2, NCOUT], F32, tag="fc2sb", name="fc2sb")
                nc.sync.dma_start(state["fc2sb"][:100, :, :], fc2_d[:].rearrange("m p o -> p m o"))
                state["fc2bsb"] = wpool.tile([128, 2], F32, tag="fc2bsb", name="fc2bsb")
                nc.sync.dma_start(state["fc2bsb"][:100, :], fc2b_d[:].rearrange("m p -> p m"))

            class Conv1Weave:
                """channel-major 1x1 conv for a sample pair, split into
                emit()-able slices so the matmuls can be woven into the
                previous pair's attention stalls.  Produces (xcmA, xcmB),
                each [128, CC, P+1] bf16 with a ones column at [.., P]."""

                def __init__(self, feat):
                    self.feat = feat
                    self.xcms = []
                    for v in range(2):
                        xcm = xcmp.tile([128, CC, P + 1], BF16, tag="xcm", name="xcm")
                        nc.vector.memset(xcm[:, :, P : P + 1], 1.0)
                        self.xcms.append(xcm)
                    self.thunks = []
                    for half in range(2):
                        pcs_box = {}

                        def alloc(half=half, pcs_box=pcs_box):
                            pcs_box["pcs"] = [
                                ps_conv.tile([128, 2, P], F32, tag="conv", name="pc")
                                for _ in range(2)
                            ]

                        for ks in range(0, KC, 4):
                            def mm_slice(ks=ks, half=half, pcs_box=pcs_box, alloc=alloc):
                                if ks == 0:
                                    alloc()
                                pcs = pcs_box["pcs"]
                                for k in range(ks, ks + 4):
                                    for ii in range(2):
                                        i = 2 * half + ii
                                        nc.tensor.matmul(
                                            pcs[ii][:],
                                            wpT[k][:, i * 128 : (i + 1) * 128],
                                            self.feat[k][:],
                                            start=(k == 0),
                                            stop=(k == KC - 1),
                                        )
                            self.thunks.append(mm_slice)

                        def acts(half=half, pcs_box=pcs_box):
                            pcs = pcs_box["pcs"]
                            for ii in range(2):
                                i = 2 * half + ii
                                for v in range(2):
                                    nc.scalar.activation(
                                        self.xcms[v][:, i, :P],
                                        pcs[ii][:, v, :],
                                        AF.Relu,
                                        bias=t1sb[:, i : i + 1],
                                    )
                        self.thunks.append(acts)
                    self._next = 0

                def emit(self, n):
                    for t in self.thunks[self._next : self._next + n]:
                        t()
                    self._next += n

                def emit_rest(self):
                    self.emit(len(self.thunks) - self._next)

            def transp_sample(xcm):
                """spatial-major fT [p(2 chunks), c] from channel-major xcm."""
                fT = fTp.tile([128, 2, C], BF16, tag="fT")
                for i in range(CC):
                    ptr = ps_sm.tile([128, 2, 128], BF16, tag="sm")
                    for m, (po, pn) in enumerate(PCH):
                        nc.tensor.transpose(
                            ptr[:pn, m, :],
                            xcm[:, i, po : po + pn],
                            identb[:, :],
                        )
                    nc.vector.tensor_copy(fT[:, :, i * 128 : (i + 1) * 128], ptr[:])
                return fT

            def xxt_mms(fT):
                """xxt chunks [d(chunk i), all c] in PSUM."""
                pxxts = []
                for i in range(CC):
                    px = ps_big.tile([128, C], F32, tag="big")
                    for m, (po, pn) in enumerate(PCH):
                        nc.tensor.matmul(
                            px[:],
                            fT[:pn, m, i * 128 : (i + 1) * 128],
                            fT[:pn, m, :],
                            start=(m == 0),
                            stop=(m == 1),
                        )
                    pxxts.append(px)
                return pxxts

            def xxt_mins(pxxts):
                mrow = smallp.tile([128, CC], BF16, tag="mrow")
                for i in range(CC):
                    nc.vector.tensor_reduce(
                        out=mrow[:, i : i + 1], in_=pxxts[i][:], op=ALU.min, axis=AX.X
                    )
                return mrow

            def xxt_sample(fT):
                pxxts = xxt_mms(fT)
                return pxxts, xxt_mins(pxxts)

            def shift_part1(mrow):
                """broadcast m along the free dim: transpose columns of mrow
                to a single partition row, then gpsimd partition broadcast."""
                mT1 = ps_sm.tile([1, CC, 128], BF16, tag="sm")
                for j in range(CC):
                    nc.tensor.transpose(
                        mT1[0:1, j, :], mrow[:, j : j + 1], identb[:, :]
                    )
                m_row = smallp.tile([1, C], BF16, tag="mline")
                nc.vector.tensor_copy(m_row[0:1, :], mT1[0:1, :, :])
                pmb = ps_sm.tile([128, C], F32, tag="sm")
                nc.tensor.matmul(pmb[:], ones1[0:1, :], m_row[0:1, :])
                return pmb

            def shift_part2(pxxts, pmb):
                """xs = m_c - xxt on DVE (freeing the xxt banks), then
                G = exp(xs) in bf16.  No PE work -- callers put PE filler
                after this emission."""
                mbc = mbcp.tile([128, C], F32, tag="mbc")
                nc.vector.tensor_copy(mbc[:], pmb[:])
                xs = xsp.tile([128, CC, C], F32, tag="xs")
                G = Gp.tile([128, CC, C], BF16, tag="G")
                for i in range(CC):
                    nc.vector.scalar_tensor_tensor(
                        out=xs[:, i, :],
                        in0=pxxts[i][:],
                        scalar=-1.0,
                        in1=mbc[:],
                        op0=ALU.mult,
                        op1=ALU.add,
                    )
                    nc.scalar.activation(G[:, i, :], xs[:, i, :], AF.Exp)
                return G

            def y_sample(G, xcm, ypad, v):
                """y = (E @ x) / Z into ypad interior; Z from ones column."""
                zinv = smallp.tile([128, CC], F32, tag="zinv")
                for i in range(CC):
                    py = ps_sm.tile([128, P + 1], F32, tag="sm")
                    for j in range(CC):
                        nc.tensor.matmul(
                            py[:],
                            G[:, j, i * 128 : (i + 1) * 128],
                            xcm[:, j, :],
                            start=(j == 0),
                            stop=(j == CC - 1),
                        )
                    nc.vector.reciprocal(zinv[:, i : i + 1], py[:, P : P + 1])
                    nc.scalar.activation(
                        ypad[:, i // 2, i % 2, v, 1 : 1 + H, 1 : 1 + W],
                        py[:, :P].rearrange("p (h w) -> p h w", h=H),
                        AF.Copy,
                        scale=zinv[:, i : i + 1],
                    )

            def conv3_octet(pairs, fc_hook=None, split_pool=False):
                """3x3 conv (fp8 DoubleRow over ci-chunk pairs) + BN2 + ReLU
                + spatial sum.  Each weight tile streams over all samples;
                two samples share one PSUM bank: only the very first matmul
                into a bank uses start=True (bank clear), the other sample's
                first matmul relies on has_written first-touch overwrite."""
                w3sb, t2sb = state["w3sb"], state["t2sb"]
                ng = len(pairs)
                for i in range(CC):
                    pzs = [ps_big.tile([128, 2, H, W], F32, tag="big", name=f"pz{g}") for g in range(ng)]
                    for t, (ky, kx) in enumerate(
                        (ky, kx) for ky in range(3) for kx in range(3)
                    ):
                        if fc_hook is not None and i > 0 and t == 3:
                            fc_hook(i - 1)
                        for jp in range(2):
                            for g in range(ng):
                                for v in range(2):
                                    nc.tensor.matmul(
                                        pzs[g][:, v],
                                        w3sb[:, jp, :, t, i * 128 : (i + 1) * 128],
                                        ypads[g][:, jp, :, v, ky : ky + H, kx : kx + W],
                                        start=(t == 0 and jp == 0 and v == 0),
                                        stop=(t == 8 and jp == 1),
                                        perf_mode=PM.DoubleRow,
                                        skip_group_check=True,
                                    )
                    for g in range(ng):
                        for v in range(2):
                            s = 2 * pairs[g] + v
                            zscr = zscrp.tile([128, H, W], BF16, tag="zscr")
                            nc.scalar.activation(
                                zscr[:],
                                pzs[g][:, v],
                                AF.Relu,
                                bias=t2sb[:, i : i + 1],
                                scale=float(1.0 / ALPHA),
                                accum_out=zall[:, i, s : s + 1],
                            )
                    if fc_hook is not None and i == CC - 1:
                        fc_hook(i)

            # ---------------- main pipeline over pairs
            feat_pend[1] = feat1
            wv0 = Conv1Weave(feat0)
            wv0.emit_rest()
            xcm_cur = wv0.xcms
            for p in range(n_pairs):
                xcmA, xcmB = xcm_cur
                if p + 2 < n_pairs:
                    load_pair(p + 2)
                if p == 2:
                    emit_w3_loads()
                if p == 4:
                    emit_fc_loads()
                wv = Conv1Weave(feat_pend.pop(p + 1)) if p + 1 < n_pairs else None
                ypad = ypads[p % 4]
                fTA = transp_sample(xcmA)
                pxA, mrA = xxt_sample(fTA)
                fTB = transp_sample(xcmB)        # PE filler while mins(A) land
                pmbA = shift_part1(mrA)
                GA = shift_part2(pxA, pmbA)
                pxB = xxt_mms(fTB)               # banks free as stts(A) land
                y_sample(GA, xcmA, ypad, 0)
                mrB = xxt_mins(pxB)              # after yA's recip/scale on DVE
                if wv:
                    wv.emit(5)                   # PE filler: mins(B) + exps(A)
                pmbB = shift_part1(mrB)
                GB = shift_part2(pxB, pmbB)
                if wv:
                    wv.emit_rest()               # PE filler: mbc/stt/exp(B)
                y_sample(GB, xcmB, ypad, 1)
                last_octet = p == n_pairs - 1
                if p % 4 == 3 or last_octet:
                    hook = None
                    if last_octet:
                        fc1sb = state["fc1sb"]
                        phs = [ps_sm.tile([128, n_samples], F32, tag="sm", name="ph")
                               for _ in range(2)]

                        def hook(j, phs=phs, fc1sb=fc1sb):
                            for m in range(2):
                                nc.tensor.matmul(
                                    phs[m][:100, :],
                                    fc1sb[:, j, m * 100 : (m + 1) * 100],
                                    zall[:, j, :],
                                    start=(j == 0),
                                    stop=(j == CC - 1),
                                )
                    conv3_octet(list(range((p // 4) * 4, p + 1)), fc_hook=hook,
                                split_pool=last_octet)
                xcm_cur = wv.xcms if wv else None

            # ---- FC head over the whole per-core batch
            fc1bsb = state["fc1bsb"]
            fc2sb, fc2bsb = state["fc2sb"], state["fc2bsb"]
            h_sb = smallp.tile([128, 2, n_samples], F32, tag="h")
            for m in range(2):
                nc.scalar.activation(
                    h_sb[:100, m, :], phs[m][:100, :], AF.Relu, bias=fc1bsb[:100, m : m + 1]
                )
            sf_sb = smallp.tile([128, 2, n_samples], F32, tag="sf")
            for m2 in range(2):
                psf = ps_sm.tile([128, n_samples], F32, tag="sm")
                for m in range(2):
                    nc.tensor.matmul(
                        psf[:100, :],
                        fc2sb[:100, m, m2 * 100 : (m2 + 1) * 100],
                        h_sb[:100, m, :],
                        start=(m == 0),
                        stop=(m == 1),
                    )
                nc.scalar.activation(
                    sf_sb[:100, m2, :],
                    psf[:100, :],
                    AF.Identity,
                    bias=fc2bsb[:100, m2 : m2 + 1],
                )
                nc.sync.dma_start(
                    out_d[:, m2 * 100 : (m2 + 1) * 100].rearrange("b o -> o b"),
                    sf_sb[:100, m2, :],
                )

    nc.compile()
    return nc


# ---------------------------------------------------------------- host wrapper

_prog_cache = {}


def _get_program(n_samples=BPC):
    key = n_samples
    if key not in _prog_cache:
        _prog_cache[key] = build_program(n_samples)
    return _prog_cache[key]


def _fp8_tapsum_round(wa):
    """Quantize conv3 weights [co, ci, 3, 3] to fp8 e4m3 choosing per-tap
    rounding direction so the 9-tap sum error cancels per (co, ci).  The
    global-avg-pool output depends (to first order) on sum_tap w, so this
    kills the dominant correlated quantization term."""
    fp8 = ml_dtypes.float8_e4m3
    q = wa.astype(fp8)
    qf = q.astype(np.float32)
    bits = q.view(np.uint8).astype(np.int16)
    above = np.where(qf >= 0, bits + 1, bits - 1)
    below = np.where(qf >= 0, bits - 1, bits + 1)
    tgt = np.where(qf > wa, below, above).astype(np.int16)
    tgt = np.clip(tgt, 0, 255).astype(np.uint8)
    of = tgt.view(fp8).astype(np.float32)
    of = np.where(~np.isfinite(of), qf, of)
    co, ci = wa.shape[:2]
    Qf = qf.reshape(co, ci, 9).copy()
    D = (of - qf).reshape(co, ci, 9).copy()
    r = (qf - wa).reshape(co, ci, 9).sum(-1)
    for _ in range(4):
        cand = np.abs(r[..., None] + D)
        best = cand.argmin(-1)
        bi = np.take_along_axis(D, best[..., None], axis=-1)[..., 0]
        improve = np.abs(r + bi) < np.abs(r) - 1e-9
        bsel = np.where(improve, best, -1)
        for t in range(9):
            m = bsel == t
            Qf[m, t] = Qf[m, t] + D[m, t]
            r[m] += D[m, t]
            D[m, t] = -D[m, t]
    return Qf.reshape(co, ci, 9).astype(fp8)


def prepare_host_inputs(inputs):
    """Fold BN into weights, build the per-core replicated param arrays."""
    s1 = inputs["bn1_gamma"] / np.sqrt(inputs["bn1_var"] + EPS)
    t1 = (inputs["b_reduce"] - inputs["bn1_mean"]) * s1 + inputs["bn1_beta"]
    Wp = inputs["w_reduce"].reshape(C, CIN) * s1[:, None]
    wpT = np.ascontiguousarray(Wp.T).astype(ml_dtypes.bfloat16)  # [2048, 512]
    t1b = np.ascontiguousarray(t1.astype(np.float32).reshape(CC, 128))

    s2 = inputs["bn2_gamma"] / np.sqrt(inputs["bn2_var"] + EPS)
    t2 = (inputs["b3"] - inputs["bn2_mean"]) * s2 + inputs["bn2_beta"]
    w3p = inputs["w3"] * s2[:, None, None, None]            # [co, ci, ky, kx]
    w3q = _fp8_tapsum_round(np.asarray(w3p, np.float32) * ALPHA)
    # -> [ci_in(128), jp(2), jj(2), tap(9), co] fp8, scaled by ALPHA
    w3r = w3q.reshape(C, C, 9).transpose(1, 2, 0).reshape(2, 2, 128, 9, C)
    w3b = np.ascontiguousarray(w3r.transpose(2, 0, 1, 3, 4))
    t2_a = np.ascontiguousarray(t2.reshape(CC, 128))

    fc1p = (inputs["fc1_w"] / float(P)).astype(np.float32)  # fold 1/196 mean
    fc1 = np.ascontiguousarray(fc1p.T.reshape(CC, 128, NCOUT))
    fc1b = np.ascontiguousarray(inputs["fc1_b"].reshape(2, 100))
    fc2 = np.ascontiguousarray(inputs["fc2_w"].T.reshape(2, 100, NCOUT))
    fc2b = np.ascontiguousarray(inputs["fc2_b"].reshape(2, 100))
    ident = np.eye(128, dtype=np.float32)
    identb = np.eye(128, dtype=ml_dtypes.bfloat16)
    return {
        "identb": identb,
        "wpT": wpT,
        "t1b": t1b,
        "w3b": w3b,
        "t2": t2_a,
        "fc1": fc1,
        "fc1b": fc1b,
        "fc2": fc2,
        "fc2b": fc2b,
        "ident": ident,
    }


def run(inputs, n_samples=BPC, n_cores=N_CORES, trace=False):
    nc = _get_program(n_samples)
    params = prepare_host_inputs(inputs)
    feat = np.asarray(inputs["feature"], np.float32).reshape(B, CIN, P).astype(ml_dtypes.bfloat16)
    feat = feat.reshape(B // 2, 2, KC, 128, P).transpose(0, 2, 3, 1, 4)
    npair = n_samples // 2
    in_maps = []
    for c in range(n_cores):
        m = dict(params)
        m["feat"] = np.ascontiguousarray(feat[c * npair : (c + 1) * npair])
        in_maps.append(m)
    res = run_bass_kernel_spmd(nc, in_maps, list(range(n_cores)), trace=trace)
    out = np.concatenate([res.results[c]["out"] for c in range(n_cores)], axis=0)
    return out, res


def kernel(**inputs):
    inputs = {k: np.asarray(v) for k, v in inputs.items()}
    out, _ = run(inputs)
    return out.astype(np.float32)
